# revision 40
# baseline (speedup 1.0000x reference)
"""AttentionBlock (GroupNorm + MHSA + proj + residual) on 8 TRN2 NeuronCores.

Sharding: data-parallel over batch (B=8 -> 1 batch element per core), SPMD —
one Bass program, per-core input maps.

Per-core math (C=512, T=1024, 8 heads, ch=64, 32 groups):
  h   = GroupNorm(x)                          (512, 1024)
  qkv = Wqkv h   (q,k pre-scaled by 64^-0.25 on host; q/k biases added on DVE,
                  v bias folded into b_proj on host)
  per head-pair, per t-half (nq):  S^T(s,t) = k^T q   (no max-subtraction)
             P^T = exp(S^T)  (ACT, double-buffered score PSUM)
             a   = v' P^T   (v' has a ones row per head giving the Z row)
             a  /= Z  (broadcast via tiny e2 matmul + DVE mul)
  out = Wproj a + b_proj;  y = x + out

Structure is organized so the ACT engine (exp: 64 x N=1024 instrs ~73us) is
the saturated resource; PE work (~72us) overlaps it, as do DVE/DMA.
PSUM budget (8 banks): scores f32 [128,1024] x2 bufs = 4, av accumulators
[65,512] x2 = 2, aux ring [128,512] x2 = 2.
"""

import sys
import ml_dtypes
import numpy as np

sys.path.insert(0, "/opt/trn_rl_repo")

BF16_NP = ml_dtypes.bfloat16

import concourse.bacc as bacc
import concourse.bass as bass
import concourse.mybir as mybir
import concourse.tile as tile
from concourse import bass_utils

F32 = mybir.dt.float32
F32R = mybir.dt.float32r
BF16 = mybir.dt.bfloat16
AF = mybir.ActivationFunctionType
ALU = mybir.AluOpType

B, C, HH, WW = 8, 512, 32, 32
T = HH * WW            # 1024
NH = 8                 # heads
CH = C // NH           # 64 per-head dim
NCT = C // 128         # 4 channel tiles
NTT = T // 128         # 8 seq tiles
NP = NH // 2           # 4 head pairs
SCALE = 1.0 / np.sqrt(np.sqrt(CH))
EPS = 1e-5
WQK = 2 * C            # 1024 cols of q|k section
WV = C                 # 512 cols of v section (no Z cols in W)
WQ = WQK + WV          # 1536

_CACHE = {}

# cblob column layout (f32, [128, 28]):
#   0:4   gamma per ct
#   4:8   beta per ct
#   8:16  gred (group-reduce 1/16 selector, 8 group-slots)
#   16:24 q/k biases: col 16+2p = q bias of pair p, 17+2p = k bias
#   24:28 bproj (v-bias folded in) per m-tile
CB_GAMMA = 0
CB_BETA = 4
CB_GRED = 8
CB_BQK = 16
CB_BPROJ = 24
CB_W = 28


def build_kernel(debug=False):
    nc = bacc.Bacc(
        "TRN2", target_bir_lowering=False, debug=debug, num_devices=8
    )

    x_d = nc.dram_tensor("x", (C, T), BF16, kind="ExternalInput")
    wqkvT_d = nc.dram_tensor("wqkvT", (C, WQ), BF16, kind="ExternalInput")
    wprojT_d = nc.dram_tensor("wprojT", (C, C), BF16, kind="ExternalInput")
    cblob_d = nc.dram_tensor("cblob", (128, CB_W), F32, kind="ExternalInput")
    gbcast_d = nc.dram_tensor("gbcast", (8, 128), F32, kind="ExternalInput")
    e2_d = nc.dram_tensor("e2", (2, 128), BF16, kind="ExternalInput")
    y_d = nc.dram_tensor("y", (C, T), F32, kind="ExternalOutput")

    with tile.TileContext(nc) as tc:
        with (
            tc.tile_pool(name="single", bufs=1) as single,
            tc.tile_pool(name="hp", bufs=1) as hp,
            tc.tile_pool(name="qkp", bufs=1) as qkp,
            tc.tile_pool(name="vtp", bufs=1) as vtp,
            tc.tile_pool(name="ptp", bufs=3) as ptp,
            tc.tile_pool(name="aap", bufs=1) as aap,
            tc.tile_pool(name="gnp", bufs=2) as gnp,
            tc.tile_pool(name="nrm", bufs=2) as nrm,
            tc.tile_pool(name="pp", bufs=1, space="PSUM") as pp,
        ):
            # ---------------- constant / weight / x loads ----------------
            # x first (it gates GroupNorm -> everything); 8 chunks so stats
            # can start on the first 512 columns while the rest streams.
            cblob = single.tile([128, CB_W], F32, tag="cblob")
            nc.sync.dma_start(out=cblob[:, :], in_=cblob_d.ap())
            gbcast = single.tile([8, 128], F32, tag="gbcast")
            nc.sync.dma_start(out=gbcast[:, :], in_=gbcast_d.ap())
            e2 = single.tile([2, 128], BF16, tag="e2")
            nc.sync.dma_start(out=e2[:, :], in_=e2_d.ap())

            # x chunk DMAs are descriptor-bound (one descriptor per
            # partition-row); slice each chunk across partition quarters so
            # they spread over many DMA engines instead of serializing on one.
            xbig = single.tile([128, NCT, T], BF16, tag="xbig")
            xr4 = x_d.ap().rearrange("(c p) (s f) -> p c s f", p=128, f=512)
            for ct in range(NCT):
                for sg in range(2):
                    nc.sync.dma_start(
                        out=xbig[:, ct, sg * 512:(sg + 1) * 512],
                        in_=xr4[:, ct, sg, :],
                    )
            x_t = [xbig[:, ct, :] for ct in range(NCT)]
            ybig = single.tile([128, NCT, T], F32, tag="ybig")
            y_t = [ybig[:, ct, :] for ct in range(NCT)]

            wqbig = single.tile([128, NCT, WQ], BF16, tag="wqbig")
            nc.sync.dma_start(
                out=wqbig[:, :, :],
                in_=wqkvT_d.ap().rearrange("(c p) t -> p c t", p=128),
            )
            wq_t = [wqbig[:, ct, :] for ct in range(NCT)]
            wpbig = single.tile([128, NCT, C], BF16, tag="wpbig")
            nc.sync.dma_start(
                out=wpbig[:, :, :],
                in_=wprojT_d.ap().rearrange("(c p) t -> p c t", p=128),
            )
            wp_t = [wpbig[:, ct, :] for ct in range(NCT)]

            # Prefetch the Exp table set (the only ACT set this kernel uses)
            # while DMAs stream, so the first attention exp pays no load.
            warm = gnp.tile([1, 1], F32, tag="warm")
            nc.scalar.activation(out=warm[:, :], in_=cblob[0:1, 0:1], func=AF.Exp)

            # ---------------- GroupNorm ----------------
            # per-channel stats via bn_stats/bn_aggr; cross-channel (16/group)
            # reduction + broadcast via tiny matmuls through the aux psum ring.
            # cs cols 2ct: mean, 2ct+1: E[x^2]
            cs = gnp.tile([128, 2 * NCT], F32, tag="cs")
            for ct in range(NCT):
                xr = x_t[ct][:, :].rearrange("p (n f) -> p n f", f=512)
                st = gnp.tile([128, 2, 6], F32, tag="st")
                for sg in range(2):
                    nc.vector.bn_stats(out=st[:, sg, :], in_=xr[:, sg, :])
                nc.vector.bn_aggr(out=cs[:, 2 * ct:2 * ct + 2], in_=st[:, :, :])
                # E[x^2] = mean*mean + var  (in place over the var column)
                nc.vector.scalar_tensor_tensor(
                    out=cs[:, 2 * ct + 1:2 * ct + 2],
                    in0=cs[:, 2 * ct:2 * ct + 1],
                    scalar=cs[:, 2 * ct:2 * ct + 1],
                    in1=cs[:, 2 * ct + 1:2 * ct + 2],
                    op0=ALU.mult, op1=ALU.add,
                )
            # HAM warmup: ~3.5us of back-to-back dummy matmuls ending just
            # before the qkv matmuls start, so they run at 2.4GHz instead of
            # the cold 1.2GHz (the PE is otherwise idle >3.4us during GN and
            # would start throttled).
            wps = pp.tile([128, 128], F32, tag="aux", bufs=2, name="wps")
            for wi in range(48):
                nc.tensor.matmul(
                    wps[:, :], xbig[:, 3, 896:1024], xbig[:, 3, 0:128],
                    start=True, stop=True,
                )

            gsp = pp.tile([8, 2 * NCT], F32, tag="aux", bufs=2, name="gsp")
            nc.tensor.matmul(
                gsp[:, :], cblob[:, CB_GRED:CB_GRED + 8], cs[:, :],
                start=True, stop=True,
            )
            # gs cols 2ct: group mean, 2ct+1: group rstd (after ln/exp)
            gs = gnp.tile([8, 2 * NCT], F32, tag="gs")
            nc.vector.tensor_copy(out=gs[:, :], in_=gsp[:, :])
            t1 = gnp.tile([8, NCT], F32, tag="t1")
            nc.vector.tensor_mul(
                out=t1[:, :], in0=gs[:, 0::2], in1=gs[:, 0::2]
            )
            veps = gnp.tile([8, NCT], F32, tag="veps")
            nc.vector.scalar_tensor_tensor(
                out=veps[:, :], in0=gs[:, 1::2], scalar=EPS, in1=t1[:, :],
                op0=ALU.add, op1=ALU.subtract,
            )
            # rstd = 1/sqrt(var+eps) on DVE only: bit-trick seed + 2 Newton
            # steps (keeps the ACT engine exp-only -> single table set).
            iv = gnp.tile([8, NCT], mybir.dt.int32, tag="iv")
            nc.vector.tensor_scalar(
                out=iv[:, :], in0=veps[:, :].bitcast(mybir.dt.int32),
                scalar1=1, scalar2=None, op0=ALU.arith_shift_right,
            )
            nc.vector.tensor_scalar(
                out=iv[:, :], in0=iv[:, :], scalar1=0x5F3759DF, scalar2=-1,
                op0=ALU.subtract, op1=ALU.mult,
            )
            y0 = iv[:, :].bitcast(F32)
            tn = gnp.tile([8, NCT], F32, tag="tn")
            for _ in range(2):
                nc.vector.tensor_mul(out=tn[:, :], in0=veps[:, :], in1=y0)
                nc.vector.tensor_mul(out=tn[:, :], in0=tn[:, :], in1=y0)
                nc.vector.tensor_scalar(
                    out=tn[:, :], in0=tn[:, :], scalar1=-0.5, scalar2=1.5,
                    op0=ALU.mult, op1=ALU.add,
                )
                nc.vector.tensor_mul(out=iv[:, :].bitcast(F32), in0=y0, in1=tn[:, :])
            nc.vector.tensor_copy(out=gs[:, 1::2], in_=y0)
            h_t = []
            for ct in range(NCT):
                mrc = pp.tile([128, 2], F32, tag="aux", bufs=2, name=f"mrc{ct}")
                nc.tensor.matmul(
                    mrc[:, :], gbcast[:, :], gs[:, 2 * ct:2 * ct + 2],
                    start=True, stop=True,
                )
                sc = gnp.tile([128, 1], F32, tag="sc")
                sh = gnp.tile([128, 1], F32, tag="sh")
                nc.vector.tensor_mul(
                    out=sc[:, :], in0=mrc[:, 1:2],
                    in1=cblob[:, CB_GAMMA + ct:CB_GAMMA + ct + 1],
                )
                nc.vector.tensor_mul(out=sh[:, :], in0=mrc[:, 0:1], in1=sc[:, :])
                nc.vector.tensor_sub(
                    out=sh[:, :],
                    in0=cblob[:, CB_BETA + ct:CB_BETA + ct + 1], in1=sh[:, :],
                )
                ht = hp.tile([128, T], BF16, tag=f"h{ct}", name=f"h{ct}")
                nc.vector.tensor_scalar(
                    out=ht[:, :], in0=x_t[ct][:, :], scalar1=sc[:, :],
                    scalar2=sh[:, :], op0=ALU.mult, op1=ALU.add,
                )
                h_t.append(ht)

            # ---------------- q/k for all pairs (SBUF, bf16) ----------------
            # W col layout per pair p: [q(2p)|q(2p+1)|k(2p)|k(2p+1)] in
            # cols 256p..256p+256; v section at cols 2C..2C+512.
            # Emitted as 4 chunk closures (q/k x t-half) so they can be woven
            # into the previous pair's st-loop, filling PE exp-wait slack
            # instead of stalling the pair boundary.
            def make_qk(p):
                dsts = [
                    qkp.tile([128, T], BF16, tag=f"qk{p}_{i}", name=f"qk{p}_{i}")
                    for i in range(2)
                ]

                def chunk(nq, i):
                    def go():
                        off = 256 * p + 128 * i
                        ps = pp.tile(
                            [128, 512], F32, tag="aux", bufs=2,
                            name=f"qkp{p}_{i}_{nq}",
                        )
                        for ctk in range(NCT):
                            nc.tensor.matmul(
                                ps[:, :],
                                wq_t[ctk][:, off:off + 128],
                                h_t[ctk][:, nq * 512:(nq + 1) * 512],
                                start=(ctk == 0), stop=(ctk == NCT - 1),
                            )
                        bias = cblob[:, CB_BQK + 2 * p + i:CB_BQK + 2 * p + i + 1]
                        if p == 0:
                            # ACT is idle before the first exp; bias-add there
                            # keeps the DVE free for GN/h/vt on the startup
                            # critical path.
                            nc.scalar.add(
                                out=dsts[i][:, nq * 512:(nq + 1) * 512],
                                in_=ps[:, :], add=bias,
                            )
                        else:
                            nc.vector.tensor_scalar_add(
                                out=dsts[i][:, nq * 512:(nq + 1) * 512],
                                in0=ps[:, :], scalar1=bias,
                            )
                    return go

                return dsts, [chunk(0, 0), chunk(0, 1), chunk(1, 0), chunk(1, 1)]

            qk_s = {}
            qk_chunks = {}
            for p in range(NP):
                qk_s[p], qk_chunks[p] = make_qk(p)
            for go in qk_chunks[0]:
                go()

            # ---------------- v^T tiles ----------------
            # vt[tt] layout [128 (t), 8 heads, 65]: cols 0..63 = v channels,
            # col 64 = ones (Z row source) via memset.
            vt_t = [
                vtp.tile([128, NH, CH + 1], BF16, tag=f"vt{tt}", name=f"vt{tt}")
                for tt in range(NTT)
            ]

            def make_v(tt):
                def go():
                    vps = pp.tile(
                        [128, 512], F32, tag="aux", bufs=2, name=f"vps{tt}"
                    )
                    for ctk in range(NCT):
                        nc.tensor.matmul(
                            vps[:, :],
                            h_t[ctk][:, tt * 128:(tt + 1) * 128],
                            wq_t[ctk][:, WQK:WQK + WV],
                            start=(ctk == 0), stop=(ctk == NCT - 1),
                        )
                    vt = vt_t[tt]
                    nc.vector.tensor_copy(
                        out=vt[:, :, 0:CH],
                        in_=vps[:, :].rearrange("p (h c) -> p h c", h=NH),
                    )
                    nc.vector.memset(vt[:, :, CH:CH + 1], 1.0)
                return go

            make_v(0)()
            make_v(1)()

            # ---------------- attention + per-(pair,nq) normalize ----------------
            aunbig = single.tile([CH + 1, NH, T], F32, tag="aunbig")
            a_all = [
                aap.tile([128, T], BF16, tag=f"aall{ct}", name=f"aall{ct}")
                for ct in range(NCT)
            ]

            def emit_zchain(p, nq):
                # DVE/DMA-only part of softmax normalization: gather Z rows,
                # reciprocal, cast.  Emitted right after the aunbig copies of
                # (p, nq); the broadcast/mul is deferred (emit_zmul) to a
                # point where invz is long ready, so the in-order PE queue
                # never waits on this chain.
                # Pair 3 (latency-critical): one combined [2,512] chain (one
                # DMA + one reciprocal), consumed by partition-matched K=1 PE
                # matmuls.  Other pairs: per-head [1,512] tiles at partition 0
                # for the GPSIMD broadcast path.
                sl = slice(nq * 512, (nq + 1) * 512)
                invz = []
                for hl in range(2):
                    zp = nrm.tile(
                        [1, 512], F32, tag="zp", bufs=4, name=f"zp{p}_{nq}_{hl}"
                    )
                    nc.sync.dma_start(
                        out=zp[:, :],
                        in_=aunbig[CH:CH + 1, 2 * p + hl:2 * p + hl + 1, sl],
                    )
                    invzf = nrm.tile(
                        [1, 512], F32, tag="invzf", bufs=4,
                        name=f"invzf{p}_{nq}_{hl}",
                    )
                    nc.vector.reciprocal_approx_fast(out=invzf[:, :], in_=zp[:, :])
                    iz = nrm.tile(
                        [1, 512], BF16, tag="invz", bufs=8, name=f"invz{p}_{nq}_{hl}"
                    )
                    nc.vector.tensor_copy(out=iz[:, :], in_=invzf[:, :])
                    invz.append(iz)
                return invz

            invz_t = {}

            def emit_zmul(p, nq, tail=False):
                # Normalize: broadcast 1/Z to 64 rows per head, multiply.
                # Steady state: broadcast AND multiply on the (idle) GPSIMD
                # engine, keeping both the PE queue and the congested DVE free.
                # Tail (last pair's second half): broadcast via a K=1 PE
                # matmul (PE is warm+idle there) and multiply on DVE — the
                # lowest-latency path.
                sl = slice(nq * 512, (nq + 1) * 512)
                invz = invz_t[(p, nq)]
                atmp = nrm.tile([CH, 512], BF16, tag="atmp", name=f"atmp{p}_{nq}")
                for hl in range(2):
                    h_ = 2 * p + hl
                    dst = a_all[p][0:CH, sl] if hl == 0 else atmp[:, :]
                    if tail:
                        zb = pp.tile(
                            [CH, 512], F32, tag="aux", bufs=2, name=f"zb{h_}_{nq}"
                        )
                        nc.tensor.matmul(
                            zb[:, :], e2[0:1, 0:CH], invz[hl][:, :],
                            start=True, stop=True,
                        )
                        nc.vector.tensor_mul(
                            out=dst, in0=aunbig[0:CH, h_, sl], in1=zb[:, :]
                        )
                    else:
                        zbs = nrm.tile(
                            [CH, 512], BF16, tag="zbs", bufs=4, name=f"zbs{h_}_{nq}"
                        )
                        nc.gpsimd.partition_broadcast(
                            out_ap=zbs[:, :], in_ap=invz[hl][0:1, :]
                        )
                        nc.vector.tensor_mul(
                            out=dst, in0=aunbig[0:CH, h_, sl], in1=zbs[:, :]
                        )
                nc.sync.dma_start(out=a_all[p][CH:2 * CH, sl], in_=atmp[:, :])

            def emit_proj(m, nq):
                pps = pp.tile(
                    [128, 512], F32, tag="aux", bufs=2, name=f"pj{m}_{nq}"
                )
                for ck in range(NCT):
                    nc.tensor.matmul(
                        pps[:, :],
                        wp_t[ck][:, m * 128:(m + 1) * 128],
                        a_all[ck][:, nq * 512:(nq + 1) * 512],
                        start=(ck == 0), stop=(ck == NCT - 1),
                    )
                nc.vector.scalar_tensor_tensor(
                    out=y_t[m][:, nq * 512:(nq + 1) * 512],
                    in0=pps[:, :],
                    scalar=cblob[:, CB_BPROJ + m:CB_BPROJ + m + 1],
                    in1=x_t[m][:, nq * 512:(nq + 1) * 512],
                    op0=ALU.add, op1=ALU.add,
                )
                nc.sync.dma_start(
                    out=y_d.ap()[m * 128:(m + 1) * 128, nq * 512:(nq + 1) * 512],
                    in_=y_t[m][:, nq * 512:(nq + 1) * 512],
                )

            def emit_pair(p, inserts=None):
                qs, ks = qk_s[p]
                for nq in range(2):
                    avps = [
                        pp.tile([CH + 1, 512], F32, tag=f"av{hl}", bufs=1,
                                name=f"av{p}_{nq}_{hl}")
                        for hl in range(2)
                    ]
                    for st_ in range(NTT):
                        scps = pp.tile(
                            [128, T], F32, tag="sc", bufs=2,
                            name=f"sc{p}_{nq}_{st_}",
                        )
                        for hl in range(2):
                            base = 64 * hl
                            nc.tensor.matmul(
                                scps[:, hl * 512:(hl + 1) * 512],
                                ks[base:base + 64, st_ * 128:(st_ + 1) * 128],
                                qs[base:base + 64, nq * 512:(nq + 1) * 512],
                                start=True, stop=True,
                            )
                        pt = ptp.tile([128, T], BF16, tag="pt", name=f"pt{p}_{nq}_{st_}")
                        nc.scalar.activation(
                            out=pt[:, :], in_=scps[:, :], func=AF.Exp
                        )
                        for hl in range(2):
                            h_ = 2 * p + hl
                            nc.tensor.matmul(
                                avps[hl][:, :],
                                vt_t[st_][:, h_, :],
                                pt[:, hl * 512:(hl + 1) * 512],
                                start=(st_ == 0), stop=(st_ == NTT - 1),
                            )
                        if inserts:
                            for fn in inserts.get((nq, st_), ()):
                                fn()
                    sl = slice(nq * 512, (nq + 1) * 512)
                    if p == 3:
                        # Z rows first so the zchain gather DMA fires before
                        # the bulk copies — shortens the tail latency chain.
                        for hl in range(2):
                            nc.vector.tensor_copy(
                                out=aunbig[CH:CH + 1, 2 * p + hl, sl],
                                in_=avps[hl][CH:CH + 1, :],
                            )
                        invz_t[(p, nq)] = emit_zchain(p, nq)
                        for hl in range(2):
                            nc.vector.tensor_copy(
                                out=aunbig[0:CH, 2 * p + hl, sl],
                                in_=avps[hl][0:CH, :],
                            )
                    else:
                        for hl in range(2):
                            nc.vector.tensor_copy(
                                out=aunbig[:, 2 * p + hl, sl],
                                in_=avps[hl][:, :],
                            )
                        invz_t[(p, nq)] = emit_zchain(p, nq)

            # Schedule: every non-attention PE op is woven into an st-loop
            # position where its inputs are already ready, so the in-order PE
            # queue never waits and the ACT engine stays exp-saturated.
            def qk_inserts(pn):
                return {(1, 2 * j): [qk_chunks[pn][j]] for j in range(4)}

            def zmul_inserts(pz):
                return {
                    (0, 1): [lambda: emit_zmul(pz, 0)],
                    (0, 3): [lambda: emit_zmul(pz, 1)],
                }

            ins0 = {(0, j): [make_v(2 + j)] for j in range(6)}
            ins0.update(qk_inserts(1))
            emit_pair(0, inserts=ins0)
            ins1 = qk_inserts(2)
            ins1.update(zmul_inserts(0))
            emit_pair(1, inserts=ins1)
            ins2 = qk_inserts(3)
            ins2.update(zmul_inserts(1))
            emit_pair(2, inserts=ins2)
            # Pair 3: weave in pair-2 normalize, then its own nq0 normalize +
            # nq0 projection, so only nq1's normalize + projection tail out.
            ins3 = zmul_inserts(2)
            ins3.update({
                (1, 2): [lambda: emit_zmul(3, 0, tail=True)],
                (1, 3): [lambda: emit_proj(0, 0)],
                (1, 4): [lambda: emit_proj(1, 0)],
                (1, 5): [lambda: emit_proj(2, 0)],
                (1, 6): [lambda: emit_proj(3, 0)],
            })
            emit_pair(3, inserts=ins3)
            # nq1 projection: pre-accumulate the pair 0-2 contributions right
            # after the last AV (PE would otherwise idle into a HAM
            # re-throttle), then only 4 matmuls + stt + DMA follow the final
            # normalize.
            # Keep the PE busy through the final normalize chain so the last
            # projection runs warm (a >3.4us idle would re-throttle to
            # 1.2GHz).
            wps2 = pp.tile([128, 128], F32, tag="aux", bufs=2, name="wps2")
            for wi in range(80):
                nc.tensor.matmul(
                    wps2[:, :], wqbig[:, 0, 0:128], wqbig[:, 0, 128:256],
                    start=True, stop=True,
                )
            emit_zmul(3, 1, tail=True)
            for m in range(NCT):
                emit_proj(m, 1)

    nc.compile()
    return nc


def make_in_maps(x, gn_weight, gn_bias, w_qkv, b_qkv, w_proj, b_proj):
    x = np.asarray(x, dtype=np.float32)
    w_qkv = np.asarray(w_qkv, dtype=np.float32)
    b_qkv = np.asarray(b_qkv, dtype=np.float32)
    w_proj = np.asarray(w_proj, np.float32)
    b_proj = np.asarray(b_proj, np.float32)
    scale = np.float32(SCALE)
    wq = w_qkv.copy()
    bq = b_qkv.copy()
    for hd in range(NH):
        sl = slice(3 * CH * hd, 3 * CH * hd + 2 * CH)  # q,k rows of this head
        wq[sl] *= scale
        bq[sl] *= scale
    # Column order expected by the kernel: per head-pair p the contiguous
    # blocks [q(2p) | q(2p+1) | k(2p) | k(2p+1)] (256 cols each), then all
    # v blocks (64 per head, no Z columns).
    perm = []
    for p in range(NP):
        for hd in (2 * p, 2 * p + 1):
            perm.extend(range(3 * CH * hd, 3 * CH * hd + CH))          # q
        for hd in (2 * p, 2 * p + 1):
            perm.extend(range(3 * CH * hd + CH, 3 * CH * hd + 2 * CH))  # k
    for hd in range(NH):
        perm.extend(range(3 * CH * hd + 2 * CH, 3 * CH * hd + 3 * CH))  # v
    perm = np.asarray(perm)
    wq = wq[perm]
    bq = bq[perm]
    wqkvT = np.ascontiguousarray(wq.T).astype(BF16_NP)  # (C, 1536)

    # v bias folded through the projection: out += Wproj @ b_v
    bv_full = np.empty(C, np.float32)
    for hd in range(NH):
        bv_full[hd * CH:(hd + 1) * CH] = b_qkv[3 * CH * hd + 2 * CH:3 * CH * hd + 3 * CH]
    bproj2 = b_proj + w_proj @ bv_full

    wprojT = np.ascontiguousarray(w_proj.T).astype(BF16_NP)  # (C, C)
    gamma = np.asarray(gn_weight, np.float32).reshape(NCT, 128).T
    beta = np.asarray(gn_bias, np.float32).reshape(NCT, 128).T
    gred = np.zeros((128, 8), np.float32)
    gbcast = np.zeros((8, 128), np.float32)
    for c in range(128):
        gred[c, c // 16] = 1.0 / 16.0
        gbcast[c // 16, c] = 1.0
    bqk = np.zeros((128, 8), np.float32)
    for p in range(NP):
        bqk[:, 2 * p] = bq[256 * p:256 * p + 128]
        bqk[:, 2 * p + 1] = bq[256 * p + 128:256 * p + 256]
    bproj_col = bproj2.reshape(NCT, 128).T
    cblob = np.ascontiguousarray(
        np.concatenate([gamma, beta, gred, bqk, bproj_col], axis=1)
    )  # (128, 28)
    e2 = np.zeros((2, 128), BF16_NP)
    e2[0, 0:CH] = 1.0
    e2[1, CH:2 * CH] = 1.0

    common = dict(
        wqkvT=wqkvT, wprojT=wprojT, cblob=cblob, gbcast=gbcast, e2=e2,
    )
    in_maps = []
    for b in range(B):
        m = dict(common)
        m["x"] = np.ascontiguousarray(x[b].reshape(C, T)).astype(BF16_NP)
        in_maps.append(m)
    return in_maps


def kernel(x, gn_weight, gn_bias, w_qkv, b_qkv, w_proj, b_proj, _trace=False):
    if "nc" not in _CACHE:
        _CACHE["nc"] = build_kernel()
    nc = _CACHE["nc"]
    in_maps = make_in_maps(x, gn_weight, gn_bias, w_qkv, b_qkv, w_proj, b_proj)
    res = bass_utils.run_bass_kernel_spmd(
        nc, in_maps, core_ids=list(range(B)), trace=_trace
    )
    out = np.stack([r["y"].reshape(C, HH, WW) for r in res.results], axis=0)
    if _trace:
        _CACHE["last_result"] = res
    return out
